# revision 19
# baseline (speedup 1.0000x reference)
"""Trainium2 Bass kernel for CustomPositionsPiecewiseConv2d.

Math: for knots positions=[-1,-.5,0,.5,1] and inputs x in [0,1], the active
interpolation coefficients are
    c2 = relu(1-2v),  c4 = max(relu(2v-1), T),  c3 = 1 - c2 - c4
with T = 1[v >= theta] the isclose(v,1) mask.  Since c2+c3+c4 == 1 exactly
(everywhere, including the zero-padding border), the c3 plane folds away:
    out = C2 (x) (W2-W3) + C4 (x) (W4-W3) + sum_ck W3[o,c,k] + bias
Each plane is an elementwise function of v and v is shifted/padded copies of x,
so planes are computed once per padded image and the 3x3 im2col becomes 9
shifted access-pattern reads feeding PSUM-accumulated matmuls.

Modes:
  float32       exact, 4 cyc/row on PE
  float32r      tf32-rounded operands, 1 cyc/row (err ~1.6e-4)
  float32r_split  hi/lo tf32 split of coeffs+weights; per tap one K=128 matmul
                  [c2h,c4h,c2l,c4l]x[W2h,W4h,W2h,W4h] plus one K=64 matmul
                  [c2h,c4h]x[W2l,W4l]; drops only (lo x lo) terms ~2^-24.

Sharding: data-parallel over batch, 2 images per core on 8 cores.
"""

import numpy as np

B, C, H, W = 16, 32, 64, 64
O, P, KH, KW = 128, 5, 3, 3
NCORES = 8
IPC = B // NCORES            # images per core
HP, WP = H + 2, W + 2        # padded image (pad=1)
RT = 8                       # output rows per L-tile
NT = H // RT                 # L-tiles per image
K2 = KH * KW
L = H * W
ATOL = 1e-5
RTOL = 1e-5

MODE = "bf16_split"          # float32 | float32r | bf16_split


# ---------------------------------------------------------------- host math


def _isclose_np(a, b):
    return np.abs(a - b) <= np.float32(ATOL) + np.float32(RTOL) * np.abs(b)


def _reference_np(x, weights, bias, positions):
    """Direct numpy port of the reference (fallback path)."""
    EPS = 1e-6
    Bn, Cn, Hn, Wn = x.shape
    On, _, Pn, KHn, KWn = weights.shape
    xp = np.pad(x, ((0, 0), (0, 0), (1, 1), (1, 1)))
    cols = [
        xp[:, :, i : i + Hn, j : j + Wn] for i in range(KHn) for j in range(KWn)
    ]
    pat = np.stack(cols, axis=2)
    v = pat.reshape(Bn, Cn, KHn * KWn, Hn * Wn).astype(np.float32)

    left, right = positions[:-1], positions[1:]
    denom = right - left
    denom = np.where(denom == 0, np.float32(EPS), denom)
    varc = (1.0 / denom).astype(np.float32)
    const = (-left * varc).astype(np.float32)

    m_first = _isclose_np(v, positions[0])
    m_last = _isclose_np(v, positions[-1])
    in_range = (~(m_first | m_last)) & (v >= positions[0]) & (v <= positions[-1])

    coeff = np.zeros(v.shape + (Pn,), np.float32)
    coeff[..., 0] += m_first.astype(np.float32)
    coeff[..., Pn - 1] += m_last.astype(np.float32)
    for p in range(Pn - 1):
        m = (in_range & (v >= positions[p]) & (v < positions[p + 1])).astype(
            np.float32
        )
        t = v * varc[p] + const[p]
        coeff[..., p] += m * (1.0 - t)
        coeff[..., p + 1] += m * t

    Wk = np.transpose(weights, (0, 1, 3, 4, 2)).reshape(On, Cn, KHn * KWn, Pn)
    ident = np.all(np.abs(Wk - 1.0) <= np.float32(ATOL + RTOL), axis=-1)
    Wk_eff = np.where(ident[..., None], np.float32(0.0), Wk)

    out = np.einsum("bcklp,ockp->bol", coeff, Wk_eff, optimize=True)
    out = out + np.einsum(
        "bckl,ock->bol", v, ident.astype(np.float32), optimize=True
    )
    out = out + bias[None, :, None]
    return out.reshape(Bn, On, Hn, Wn).astype(np.float32)


def _compute_theta():
    """Smallest fp32 v such that fp32(1-v) <= fp32(ATOL + RTOL*1.0), matching
    the reference's m_last = isclose(v, 1.0) for v <= 1."""
    tau = np.float32(np.float32(ATOL) + np.float32(RTOL) * np.float32(1.0))
    th = np.float32(np.float32(1.0) - tau)
    while np.float32(np.float32(1.0) - np.nextafter(th, np.float32(0.0))) <= tau:
        th = np.nextafter(th, np.float32(0.0))
    while np.float32(np.float32(1.0) - th) > tau:
        th = np.nextafter(th, np.float32(2.0))
    return np.float32(th)


def _host_weights(weights, bias):
    """Fold c3 away.  Returns (wfold [2C, K2, O] f32 = [W2-W3; W4-W3],
    bias_eff [O] f32 = bias + sum_ck W3, ident_any)."""
    Wk = np.transpose(weights, (0, 1, 3, 4, 2)).reshape(O, C, K2, P)
    ident = np.all(np.abs(Wk - 1.0) <= np.float32(ATOL + RTOL), axis=-1)
    ident_any = bool(ident.any())
    Wk_eff = np.where(ident[..., None], np.float32(0.0), Wk)
    W2 = Wk_eff[:, :, :, 2].astype(np.float64)
    W3 = Wk_eff[:, :, :, 3].astype(np.float64)
    W4 = Wk_eff[:, :, :, 4].astype(np.float64)
    wfold = np.zeros((2 * C, K2, O), np.float32)
    wfold[0:C] = (W2 - W3).astype(np.float32).transpose(1, 2, 0)
    wfold[C : 2 * C] = (W4 - W3).astype(np.float32).transpose(1, 2, 0)
    bias_eff = (bias.astype(np.float64) + W3.sum(axis=(1, 2))).astype(np.float32)
    return np.ascontiguousarray(wfold), np.ascontiguousarray(bias_eff), ident_any


# ---------------------------------------------------------------- device IR


def _build_nc(theta, mode):
    import concourse.tile as tile
    from concourse import bacc, mybir

    f32 = mybir.dt.float32
    f32r = mybir.dt.float32r
    bf16 = mybir.dt.bfloat16
    Alu = mybir.AluOpType
    Act = mybir.ActivationFunctionType
    split = mode == "bf16_split"
    if mode == "float32":
        plane_dt = f32
    elif mode == "float32r":
        plane_dt = f32r
    else:
        plane_dt = bf16

    nc = bacc.Bacc("TRN2", target_bir_lowering=False, debug=False,
                   num_devices=NCORES)
    x_d = nc.dram_tensor("x", [IPC, C, H, W], f32, kind="ExternalInput").ap()
    w_d = nc.dram_tensor("wfold", [2 * C, K2, O], f32, kind="ExternalInput").ap()
    b_d = nc.dram_tensor("bias", [O, 1], f32, kind="ExternalInput").ap()
    o_d = nc.dram_tensor("out", [IPC, O, H, W], f32, kind="ExternalOutput").ap()

    with tile.TileContext(nc) as tc:
        with (
            tc.tile_pool(name="const", bufs=1) as constp,
            tc.tile_pool(name="scratch", bufs=1) as scrp,
            tc.tile_pool(name="plane", bufs=1) as planep,
            tc.tile_pool(name="ybuf", bufs=2) as ybufp,
            tc.tile_pool(name="psum", bufs=1, space="PSUM") as psump,
            tc.tile_pool(name="osb", bufs=4) as osbp,
        ):
            # ---- weights ----
            w_sb = constp.tile([2 * C, K2, O], f32)
            nc.sync.dma_start(w_sb[:], w_d[:])
            b_sb = constp.tile([O, 1], f32)
            nc.sync.dma_start(b_sb[:], b_d[:])
            if mode == "float32":
                w_hi = w_sb
            else:
                w_hi = constp.tile([2 * C, K2, O], plane_dt)
                nc.vector.tensor_copy(w_hi[:], w_sb[:])
            if split:
                w_lo = constp.tile([2 * C, K2, O], plane_dt)
                nc.vector.tensor_tensor(w_lo[:], w_sb[:], w_hi[:], Alu.subtract)
                # lhsT1 rows: [W2h, W4h, W2h, W4h] (hi coeffs then lo coeffs)
                w_rep = constp.tile([4 * C, K2, O], plane_dt)
                nc.sync.dma_start(w_rep[0 : 2 * C], w_hi[:])
                nc.sync.dma_start(w_rep[2 * C : 4 * C], w_hi[:])
                lhs1, lhs2 = w_rep, w_lo
            else:
                lhs1, lhs2 = w_hi, None

            # ---- coefficient planes ----
            XF = scrp.tile([IPC * C, H, W], f32)      # flat x
            for i in range(IPC):
                nc.sync.dma_start(XF[i * C : (i + 1) * C], x_d[i])
            # scratch on the same partitions as each image's plane slice
            # (engine ops require equal SBUF base partitions across operands)
            RF = scrp.tile([IPC * C, H, W], f32)
            CF = scrp.tile([IPC * C, H, W], f32)

            npl = 4 if split else 2
            # plane buffers, padded layout; group order:
            #   split: [c2h, c4h, c2l, c4l]   else: [c2, c4]
            PL = [
                planep.tile([IPC * C, HP, WP], plane_dt, name=f"PL{g}")
                for g in range(npl)
            ]
            # borders: c2-like planes = 1 at v=0, everything else = 0
            # (memset rejects f32r dests; same-size bitcast to f32 is a no-op)
            for g, pl in enumerate(PL):
                bv = 1.0 if g == 0 else 0.0
                for strip in (
                    pl[:, 0, :],
                    pl[:, HP - 1, :],
                    pl[:, 1 : HP - 1, 0],
                    pl[:, 1 : HP - 1, WP - 1],
                ):
                    nc.vector.memset(
                        strip.bitcast(f32) if plane_dt == f32r else strip, bv
                    )

            def interior(pl):
                return pl[:, 1 : HP - 1, 1 : WP - 1]

            negone = constp.tile([IPC * C, 1], f32)
            nc.vector.memset(negone[:], -1.0)

            def phi_image(i):
                """Coefficient planes for image i (partitions i*C..i*C+C)."""
                s = slice(i * C, (i + 1) * C)
                xf = XF[s]
                neg = negone[s]
                if split:
                    # bf16 rounding absorbs the isclose(v,1) mask: for
                    # v >= 1-2^-9, relu(2v-1) rounds to exactly 1.0, and the
                    # lo-plane residual lands on hi weights scaled 2^-9.
                    c2h, c4h, c2l, c4l = (
                        pl[s, 1 : HP - 1, 1 : WP - 1] for pl in PL
                    )
                    rf = RF[s]
                    cf = CF[s]
                    nc.scalar.activation(rf, xf, Act.Relu, bias=neg, scale=2.0)
                    nc.vector.tensor_copy(c4h, rf)
                    nc.vector.tensor_tensor(c4l, rf, c4h, Alu.subtract)
                    nc.scalar.activation(cf, xf, Act.Relu, bias=1.0, scale=-2.0)
                    nc.scalar.activation(c2h, cf, Act.Copy)
                    nc.vector.tensor_tensor(c2l, cf, c2h, Alu.subtract)
                else:
                    c2, c4 = (pl[s, 1 : HP - 1, 1 : WP - 1] for pl in PL)
                    rf = RF[s]
                    ts = CF[s]
                    nc.vector.tensor_scalar(
                        ts, xf, float(theta), None, Alu.is_ge
                    )
                    nc.scalar.activation(rf, xf, Act.Relu, bias=neg, scale=2.0)
                    nc.vector.tensor_tensor(rf, rf, ts, Alu.max)
                    nc.vector.tensor_copy(c4, rf)
                    nc.scalar.activation(c2, xf, Act.Relu, bias=1.0, scale=-2.0)

            # ---- per-image gather + GEMM ----
            # Tap-outer loop: one LDWEIGHTS feeds 8 back-to-back matmuls
            # (same stationary operand), so drain overlaps the next fill and
            # the per-MM cost stays ~N/2.4 instead of the isolated latency.
            # All 8 L-tiles of an image accumulate in 8 PSUM banks at once.
            for i in range(IPC):
                phi_image(i)
                Y = ybufp.tile([npl * C, HP, WP], plane_dt, name="Y", tag="Y")
                s = slice(i * C, (i + 1) * C)
                for g, pl in enumerate(PL):
                    nc.sync.dma_start(Y[g * C : (g + 1) * C], pl[s])

                pss = [
                    psump.tile([O, RT * W], f32, name=f"ps{t}", tag=f"ps{t}")
                    for t in range(NT)
                ]
                for ki in range(K2):
                    kh, kw = divmod(ki, KW)
                    cols = slice(kw, kw + W)
                    last = ki == K2 - 1
                    for t in range(NT):
                        rows = slice(t * RT + kh, t * RT + kh + RT)
                        nc.tensor.matmul(
                            pss[t][:], lhs1[:, ki, :], Y[:, rows, cols],
                            start=(ki == 0), stop=(last and not split),
                        )
                    if split:
                        for t in range(NT):
                            rows = slice(t * RT + kh, t * RT + kh + RT)
                            nc.tensor.matmul(
                                pss[t][:], lhs2[:, ki, :],
                                Y[0 : 2 * C, rows, cols],
                                start=False, stop=last,
                            )
                for t in range(NT):
                    osb = osbp.tile([O, RT * W], f32, name="osb")
                    if t % 2 == 0:
                        nc.scalar.activation(
                            osb[:], pss[t][:], Act.Identity, bias=b_sb[:, 0:1],
                            scale=1.0,
                        )
                    else:
                        nc.vector.tensor_scalar(
                            osb[:], pss[t][:], b_sb[:, 0:1], None, Alu.add
                        )
                    nc.sync.dma_start(
                        o_d[i, :, t * RT : (t + 1) * RT, :],
                        osb[:].rearrange("o (r w) -> o r w", r=RT),
                    )
    nc.compile()
    return nc


# ---------------------------------------------------------------- entry


def _prep(inputs):
    x = np.ascontiguousarray(np.asarray(inputs["x"], dtype=np.float32))
    weights = np.ascontiguousarray(np.asarray(inputs["weights"], dtype=np.float32))
    bias = np.ascontiguousarray(np.asarray(inputs["bias"], dtype=np.float32))
    positions = np.ascontiguousarray(
        np.asarray(inputs["positions"], dtype=np.float32)
    )
    return x, weights, bias, positions


def _fast_path_ok(x, positions):
    expect = np.linspace(-1.0, 1.0, P, dtype=np.float32)
    return (
        x.shape == (B, C, H, W)
        and positions.shape == (P,)
        and np.array_equal(positions, expect)
        and float(x.min()) >= 0.0
        and float(x.max()) <= 1.0
    )


def kernel(**inputs):
    x, weights, bias, positions = _prep(inputs)
    if not _fast_path_ok(x, positions):
        return _reference_np(x, weights, bias, positions)

    wfold, bias_eff, ident_any = _host_weights(weights, bias)
    if ident_any:
        # identity-shortcut weights present: needs the raw-v plane; use the
        # exact fallback rather than a rarely-exercised device path
        return _reference_np(x, weights, bias, positions)

    from concourse.bass_utils import run_bass_kernel_spmd

    nc = _build_nc(_compute_theta(), MODE)
    bias2d = np.ascontiguousarray(bias_eff.reshape(O, 1))
    in_maps = [
        {"x": np.ascontiguousarray(x[i * IPC : (i + 1) * IPC]),
         "wfold": wfold, "bias": bias2d}
        for i in range(NCORES)
    ]
    res = run_bass_kernel_spmd(nc, in_maps, core_ids=list(range(NCORES)))
    out = np.concatenate([res.results[i]["out"] for i in range(NCORES)], axis=0)
    return np.ascontiguousarray(out)


# ------------------------------------------------------------ dev utilities


def _run_sim(inputs):
    """CoreSim single-core run (images 0..IPC-1) for correctness debugging."""
    from concourse.bass_interp import CoreSim

    x, weights, bias, positions = _prep(inputs)
    assert _fast_path_ok(x, positions)
    wfold, bias_eff, ident_any = _host_weights(weights, bias)
    assert not ident_any
    nc = _build_nc(_compute_theta(), MODE)
    sim = CoreSim(nc)
    sim.tensor("x")[:] = x[:IPC]
    sim.tensor("wfold")[:] = wfold
    sim.tensor("bias")[:] = bias_eff.reshape(O, 1)
    sim.simulate()
    return np.array(sim.tensor("out"))


# revision 21
# speedup vs baseline: 1.0329x; 1.0329x over previous
"""Trainium2 Bass kernel for CustomPositionsPiecewiseConv2d.

Math: for knots positions=[-1,-.5,0,.5,1] and inputs x in [0,1], the active
interpolation coefficients are
    c2 = relu(1-2v),  c4 = max(relu(2v-1), T),  c3 = 1 - c2 - c4
with T = 1[v >= theta] the isclose(v,1) mask.  Since c2+c3+c4 == 1 exactly
(everywhere, including the zero-padding border), the c3 plane folds away:
    out = C2 (x) (W2-W3) + C4 (x) (W4-W3) + sum_ck W3[o,c,k] + bias
Each plane is an elementwise function of v and v is shifted/padded copies of x,
so planes are computed once per padded image and the 3x3 im2col becomes 9
shifted access-pattern reads feeding PSUM-accumulated matmuls.

Modes:
  float32       exact, 4 cyc/row on PE
  float32r      tf32-rounded operands, 1 cyc/row (err ~1.6e-4)
  float32r_split  hi/lo tf32 split of coeffs+weights; per tap one K=128 matmul
                  [c2h,c4h,c2l,c4l]x[W2h,W4h,W2h,W4h] plus one K=64 matmul
                  [c2h,c4h]x[W2l,W4l]; drops only (lo x lo) terms ~2^-24.

Sharding: data-parallel over batch, 2 images per core on 8 cores.
"""

import numpy as np

B, C, H, W = 16, 32, 64, 64
O, P, KH, KW = 128, 5, 3, 3
NCORES = 8
IPC = B // NCORES            # images per core
HP, WP = H + 2, W + 2        # padded image (pad=1)
RT = 8                       # output rows per L-tile
NT = H // RT                 # L-tiles per image
K2 = KH * KW
L = H * W
ATOL = 1e-5
RTOL = 1e-5

MODE = "bf16_split"          # float32 | float32r | bf16_split


# ---------------------------------------------------------------- host math


def _isclose_np(a, b):
    return np.abs(a - b) <= np.float32(ATOL) + np.float32(RTOL) * np.abs(b)


def _reference_np(x, weights, bias, positions):
    """Direct numpy port of the reference (fallback path)."""
    EPS = 1e-6
    Bn, Cn, Hn, Wn = x.shape
    On, _, Pn, KHn, KWn = weights.shape
    xp = np.pad(x, ((0, 0), (0, 0), (1, 1), (1, 1)))
    cols = [
        xp[:, :, i : i + Hn, j : j + Wn] for i in range(KHn) for j in range(KWn)
    ]
    pat = np.stack(cols, axis=2)
    v = pat.reshape(Bn, Cn, KHn * KWn, Hn * Wn).astype(np.float32)

    left, right = positions[:-1], positions[1:]
    denom = right - left
    denom = np.where(denom == 0, np.float32(EPS), denom)
    varc = (1.0 / denom).astype(np.float32)
    const = (-left * varc).astype(np.float32)

    m_first = _isclose_np(v, positions[0])
    m_last = _isclose_np(v, positions[-1])
    in_range = (~(m_first | m_last)) & (v >= positions[0]) & (v <= positions[-1])

    coeff = np.zeros(v.shape + (Pn,), np.float32)
    coeff[..., 0] += m_first.astype(np.float32)
    coeff[..., Pn - 1] += m_last.astype(np.float32)
    for p in range(Pn - 1):
        m = (in_range & (v >= positions[p]) & (v < positions[p + 1])).astype(
            np.float32
        )
        t = v * varc[p] + const[p]
        coeff[..., p] += m * (1.0 - t)
        coeff[..., p + 1] += m * t

    Wk = np.transpose(weights, (0, 1, 3, 4, 2)).reshape(On, Cn, KHn * KWn, Pn)
    ident = np.all(np.abs(Wk - 1.0) <= np.float32(ATOL + RTOL), axis=-1)
    Wk_eff = np.where(ident[..., None], np.float32(0.0), Wk)

    out = np.einsum("bcklp,ockp->bol", coeff, Wk_eff, optimize=True)
    out = out + np.einsum(
        "bckl,ock->bol", v, ident.astype(np.float32), optimize=True
    )
    out = out + bias[None, :, None]
    return out.reshape(Bn, On, Hn, Wn).astype(np.float32)


def _compute_theta():
    """Smallest fp32 v such that fp32(1-v) <= fp32(ATOL + RTOL*1.0), matching
    the reference's m_last = isclose(v, 1.0) for v <= 1."""
    tau = np.float32(np.float32(ATOL) + np.float32(RTOL) * np.float32(1.0))
    th = np.float32(np.float32(1.0) - tau)
    while np.float32(np.float32(1.0) - np.nextafter(th, np.float32(0.0))) <= tau:
        th = np.nextafter(th, np.float32(0.0))
    while np.float32(np.float32(1.0) - th) > tau:
        th = np.nextafter(th, np.float32(2.0))
    return np.float32(th)


def _host_weights(weights, bias):
    """Fold c3 away.  Returns (wfold [2C, K2, O] f32 = [W2-W3; W4-W3],
    bias_eff [O] f32 = bias + sum_ck W3, ident_any)."""
    Wk = np.transpose(weights, (0, 1, 3, 4, 2)).reshape(O, C, K2, P)
    ident = np.all(np.abs(Wk - 1.0) <= np.float32(ATOL + RTOL), axis=-1)
    ident_any = bool(ident.any())
    Wk_eff = np.where(ident[..., None], np.float32(0.0), Wk)
    W2 = Wk_eff[:, :, :, 2].astype(np.float64)
    W3 = Wk_eff[:, :, :, 3].astype(np.float64)
    W4 = Wk_eff[:, :, :, 4].astype(np.float64)
    wfold = np.zeros((2 * C, K2, O), np.float32)
    wfold[0:C] = (W2 - W3).astype(np.float32).transpose(1, 2, 0)
    wfold[C : 2 * C] = (W4 - W3).astype(np.float32).transpose(1, 2, 0)
    bias_eff = (bias.astype(np.float64) + W3.sum(axis=(1, 2))).astype(np.float32)
    return np.ascontiguousarray(wfold), np.ascontiguousarray(bias_eff), ident_any


# ---------------------------------------------------------------- device IR


def _build_nc(theta, mode):
    import concourse.tile as tile
    from concourse import bacc, mybir

    f32 = mybir.dt.float32
    f32r = mybir.dt.float32r
    bf16 = mybir.dt.bfloat16
    Alu = mybir.AluOpType
    Act = mybir.ActivationFunctionType
    split = mode == "bf16_split"
    if mode == "float32":
        plane_dt = f32
    elif mode == "float32r":
        plane_dt = f32r
    else:
        plane_dt = bf16

    nc = bacc.Bacc("TRN2", target_bir_lowering=False, debug=False,
                   num_devices=NCORES)
    x_d = nc.dram_tensor("x", [IPC, C, H, W], f32, kind="ExternalInput").ap()
    w_d = nc.dram_tensor("wfold", [2 * C, K2, O], f32, kind="ExternalInput").ap()
    b_d = nc.dram_tensor("bias", [O, 1], f32, kind="ExternalInput").ap()
    o_d = nc.dram_tensor("out", [IPC, O, H, W], f32, kind="ExternalOutput").ap()

    with tile.TileContext(nc) as tc:
        with (
            tc.tile_pool(name="const", bufs=1) as constp,
            tc.tile_pool(name="scratch", bufs=1) as scrp,
            tc.tile_pool(name="plane", bufs=1) as planep,
            tc.tile_pool(name="ybuf", bufs=2) as ybufp,
            tc.tile_pool(name="psum", bufs=1, space="PSUM") as psump,
            tc.tile_pool(name="osb", bufs=4) as osbp,
        ):
            # ---- weights ----
            w_sb = constp.tile([2 * C, K2, O], f32)
            nc.sync.dma_start(w_sb[:], w_d[:])
            b_sb = constp.tile([O, 1], f32)
            nc.sync.dma_start(b_sb[:], b_d[:])
            if mode == "float32":
                w_hi = w_sb
            else:
                w_hi = constp.tile([2 * C, K2, O], plane_dt)
                nc.vector.tensor_copy(w_hi[:], w_sb[:])
            if split:
                w_lo = constp.tile([2 * C, K2, O], plane_dt)
                nc.vector.tensor_tensor(w_lo[:], w_sb[:], w_hi[:], Alu.subtract)
                # lhsT1 rows: [W2h, W4h, W2h, W4h] (hi coeffs then lo coeffs)
                w_rep = constp.tile([4 * C, K2, O], plane_dt)
                nc.sync.dma_start(w_rep[0 : 2 * C], w_hi[:])
                nc.sync.dma_start(w_rep[2 * C : 4 * C], w_hi[:])
                lhs1, lhs2 = w_rep, w_lo
            else:
                lhs1, lhs2 = w_hi, None

            # ---- coefficient planes ----
            XF = scrp.tile([IPC * C, H, W], f32)      # flat x
            for i in range(IPC):
                nc.sync.dma_start(XF[i * C : (i + 1) * C], x_d[i])
            # scratch on the same partitions as each image's plane slice
            # (engine ops require equal SBUF base partitions across operands)
            RF = scrp.tile([IPC * C, H, W], f32)
            CF = scrp.tile([IPC * C, H, W], f32)

            npl = 4 if split else 2
            # plane buffers, padded layout; group order:
            #   split: [c2h, c4h, c2l, c4l]   else: [c2, c4]
            PL = [
                planep.tile([IPC * C, HP, WP], plane_dt, name=f"PL{g}")
                for g in range(npl)
            ]
            # borders: c2-like planes = 1 at v=0, everything else = 0
            # (memset rejects f32r dests; same-size bitcast to f32 is a no-op)
            for g, pl in enumerate(PL):
                bv = 1.0 if g == 0 else 0.0
                for strip in (
                    pl[:, 0, :],
                    pl[:, HP - 1, :],
                    pl[:, 1 : HP - 1, 0],
                    pl[:, 1 : HP - 1, WP - 1],
                ):
                    nc.vector.memset(
                        strip.bitcast(f32) if plane_dt == f32r else strip, bv
                    )

            def interior(pl):
                return pl[:, 1 : HP - 1, 1 : WP - 1]

            negone = constp.tile([IPC * C, 1], f32)
            nc.vector.memset(negone[:], -1.0)

            def phi_all():
                """Coefficient planes, both images at once (64 partitions)."""
                xf = XF[:]
                neg = negone[:]
                if split:
                    # bf16 rounding absorbs the isclose(v,1) mask: for
                    # v >= 1-2^-9, relu(2v-1) rounds to exactly 1.0, and the
                    # lo-plane residual lands on hi weights scaled 2^-9.
                    c2h, c4h, c2l, c4l = (interior(pl) for pl in PL)
                    nc.scalar.activation(RF[:], xf, Act.Relu, bias=neg, scale=2.0)
                    nc.vector.tensor_copy(c4h, RF[:])
                    nc.vector.tensor_tensor(c4l, RF[:], c4h, Alu.subtract)
                    nc.scalar.activation(CF[:], xf, Act.Relu, bias=1.0, scale=-2.0)
                    nc.vector.tensor_copy(c2h, CF[:])
                    nc.vector.tensor_tensor(c2l, CF[:], c2h, Alu.subtract)
                else:
                    c2, c4 = (interior(pl) for pl in PL)
                    nc.vector.tensor_scalar(
                        CF[:], xf, float(theta), None, Alu.is_ge
                    )
                    nc.scalar.activation(RF[:], xf, Act.Relu, bias=neg, scale=2.0)
                    nc.vector.tensor_tensor(RF[:], RF[:], CF[:], Alu.max)
                    nc.vector.tensor_copy(c4, RF[:])
                    nc.scalar.activation(c2, xf, Act.Relu, bias=1.0, scale=-2.0)

            phi_all()

            # ---- per-image gather + GEMM ----
            # Tap-outer loop: one LDWEIGHTS feeds 8 back-to-back matmuls
            # (same stationary operand), so drain overlaps the next fill and
            # the per-MM cost stays ~N/2.4 instead of the isolated latency.
            # All 8 L-tiles of an image accumulate in 8 PSUM banks at once.
            for i in range(IPC):
                Y = ybufp.tile([npl * C, HP, WP], plane_dt, name="Y", tag="Y")
                s = slice(i * C, (i + 1) * C)
                for g, pl in enumerate(PL):
                    nc.sync.dma_start(Y[g * C : (g + 1) * C], pl[s])

                pss = [
                    psump.tile([O, RT * W], f32, name=f"ps{t}", tag=f"ps{t}")
                    for t in range(NT)
                ]
                for ki in range(K2):
                    kh, kw = divmod(ki, KW)
                    cols = slice(kw, kw + W)
                    last = ki == K2 - 1
                    for t in range(NT):
                        rows = slice(t * RT + kh, t * RT + kh + RT)
                        nc.tensor.matmul(
                            pss[t][:], lhs1[:, ki, :], Y[:, rows, cols],
                            start=(ki == 0), stop=(last and not split),
                        )
                    if split:
                        for t in range(NT):
                            rows = slice(t * RT + kh, t * RT + kh + RT)
                            nc.tensor.matmul(
                                pss[t][:], lhs2[:, ki, :],
                                Y[0 : 2 * C, rows, cols],
                                start=False, stop=last,
                            )
                for t in range(NT):
                    osb = osbp.tile([O, RT * W], f32, name="osb")
                    if t % 2 == 0:
                        nc.scalar.activation(
                            osb[:], pss[t][:], Act.Identity, bias=b_sb[:, 0:1],
                            scale=1.0,
                        )
                    else:
                        nc.vector.tensor_scalar(
                            osb[:], pss[t][:], b_sb[:, 0:1], None, Alu.add
                        )
                    nc.sync.dma_start(
                        o_d[i, :, t * RT : (t + 1) * RT, :],
                        osb[:].rearrange("o (r w) -> o r w", r=RT),
                    )
    nc.compile()
    return nc


# ---------------------------------------------------------------- entry


def _prep(inputs):
    x = np.ascontiguousarray(np.asarray(inputs["x"], dtype=np.float32))
    weights = np.ascontiguousarray(np.asarray(inputs["weights"], dtype=np.float32))
    bias = np.ascontiguousarray(np.asarray(inputs["bias"], dtype=np.float32))
    positions = np.ascontiguousarray(
        np.asarray(inputs["positions"], dtype=np.float32)
    )
    return x, weights, bias, positions


def _fast_path_ok(x, positions):
    expect = np.linspace(-1.0, 1.0, P, dtype=np.float32)
    return (
        x.shape == (B, C, H, W)
        and positions.shape == (P,)
        and np.array_equal(positions, expect)
        and float(x.min()) >= 0.0
        and float(x.max()) <= 1.0
    )


def kernel(**inputs):
    x, weights, bias, positions = _prep(inputs)
    if not _fast_path_ok(x, positions):
        return _reference_np(x, weights, bias, positions)

    wfold, bias_eff, ident_any = _host_weights(weights, bias)
    if ident_any:
        # identity-shortcut weights present: needs the raw-v plane; use the
        # exact fallback rather than a rarely-exercised device path
        return _reference_np(x, weights, bias, positions)

    from concourse.bass_utils import run_bass_kernel_spmd

    nc = _build_nc(_compute_theta(), MODE)
    bias2d = np.ascontiguousarray(bias_eff.reshape(O, 1))
    in_maps = [
        {"x": np.ascontiguousarray(x[i * IPC : (i + 1) * IPC]),
         "wfold": wfold, "bias": bias2d}
        for i in range(NCORES)
    ]
    res = run_bass_kernel_spmd(nc, in_maps, core_ids=list(range(NCORES)))
    out = np.concatenate([res.results[i]["out"] for i in range(NCORES)], axis=0)
    return np.ascontiguousarray(out)


# ------------------------------------------------------------ dev utilities


def _run_sim(inputs):
    """CoreSim single-core run (images 0..IPC-1) for correctness debugging."""
    from concourse.bass_interp import CoreSim

    x, weights, bias, positions = _prep(inputs)
    assert _fast_path_ok(x, positions)
    wfold, bias_eff, ident_any = _host_weights(weights, bias)
    assert not ident_any
    nc = _build_nc(_compute_theta(), MODE)
    sim = CoreSim(nc)
    sim.tensor("x")[:] = x[:IPC]
    sim.tensor("wfold")[:] = wfold
    sim.tensor("bias")[:] = bias_eff.reshape(O, 1)
    sim.simulate()
    return np.array(sim.tensor("out"))


# revision 25
# speedup vs baseline: 1.0428x; 1.0096x over previous
"""Trainium2 Bass kernel for CustomPositionsPiecewiseConv2d.

Math: for knots positions=[-1,-.5,0,.5,1] and inputs x in [0,1], the active
interpolation coefficients are
    c2 = relu(1-2v),  c4 = max(relu(2v-1), T),  c3 = 1 - c2 - c4
with T = 1[v >= theta] the isclose(v,1) mask.  Since c2+c3+c4 == 1 exactly
(everywhere, including the zero-padding border), the c3 plane folds away:
    out = C2 (x) (W2-W3) + C4 (x) (W4-W3) + sum_ck W3[o,c,k] + bias
Each plane is an elementwise function of v and v is shifted/padded copies of x,
so planes are computed once per padded image and the 3x3 im2col becomes 9
shifted access-pattern reads feeding PSUM-accumulated matmuls.

Modes:
  float32       exact, 4 cyc/row on PE
  float32r      tf32-rounded operands, 1 cyc/row (err ~1.6e-4)
  float32r_split  hi/lo tf32 split of coeffs+weights; per tap one K=128 matmul
                  [c2h,c4h,c2l,c4l]x[W2h,W4h,W2h,W4h] plus one K=64 matmul
                  [c2h,c4h]x[W2l,W4l]; drops only (lo x lo) terms ~2^-24.

Sharding: data-parallel over batch, 2 images per core on 8 cores.
"""

import numpy as np

B, C, H, W = 16, 32, 64, 64
O, P, KH, KW = 128, 5, 3, 3
NCORES = 8
IPC = B // NCORES            # images per core
HP, WP = H + 2, W + 2        # padded image (pad=1)
RT = 8                       # output rows per L-tile
NT = H // RT                 # L-tiles per image
K2 = KH * KW
L = H * W
ATOL = 1e-5
RTOL = 1e-5

MODE = "bf16_split"          # float32 | float32r | bf16_split


# ---------------------------------------------------------------- host math


def _isclose_np(a, b):
    return np.abs(a - b) <= np.float32(ATOL) + np.float32(RTOL) * np.abs(b)


def _reference_np(x, weights, bias, positions):
    """Direct numpy port of the reference (fallback path)."""
    EPS = 1e-6
    Bn, Cn, Hn, Wn = x.shape
    On, _, Pn, KHn, KWn = weights.shape
    xp = np.pad(x, ((0, 0), (0, 0), (1, 1), (1, 1)))
    cols = [
        xp[:, :, i : i + Hn, j : j + Wn] for i in range(KHn) for j in range(KWn)
    ]
    pat = np.stack(cols, axis=2)
    v = pat.reshape(Bn, Cn, KHn * KWn, Hn * Wn).astype(np.float32)

    left, right = positions[:-1], positions[1:]
    denom = right - left
    denom = np.where(denom == 0, np.float32(EPS), denom)
    varc = (1.0 / denom).astype(np.float32)
    const = (-left * varc).astype(np.float32)

    m_first = _isclose_np(v, positions[0])
    m_last = _isclose_np(v, positions[-1])
    in_range = (~(m_first | m_last)) & (v >= positions[0]) & (v <= positions[-1])

    coeff = np.zeros(v.shape + (Pn,), np.float32)
    coeff[..., 0] += m_first.astype(np.float32)
    coeff[..., Pn - 1] += m_last.astype(np.float32)
    for p in range(Pn - 1):
        m = (in_range & (v >= positions[p]) & (v < positions[p + 1])).astype(
            np.float32
        )
        t = v * varc[p] + const[p]
        coeff[..., p] += m * (1.0 - t)
        coeff[..., p + 1] += m * t

    Wk = np.transpose(weights, (0, 1, 3, 4, 2)).reshape(On, Cn, KHn * KWn, Pn)
    ident = np.all(np.abs(Wk - 1.0) <= np.float32(ATOL + RTOL), axis=-1)
    Wk_eff = np.where(ident[..., None], np.float32(0.0), Wk)

    out = np.einsum("bcklp,ockp->bol", coeff, Wk_eff, optimize=True)
    out = out + np.einsum(
        "bckl,ock->bol", v, ident.astype(np.float32), optimize=True
    )
    out = out + bias[None, :, None]
    return out.reshape(Bn, On, Hn, Wn).astype(np.float32)


def _compute_theta():
    """Smallest fp32 v such that fp32(1-v) <= fp32(ATOL + RTOL*1.0), matching
    the reference's m_last = isclose(v, 1.0) for v <= 1."""
    tau = np.float32(np.float32(ATOL) + np.float32(RTOL) * np.float32(1.0))
    th = np.float32(np.float32(1.0) - tau)
    while np.float32(np.float32(1.0) - np.nextafter(th, np.float32(0.0))) <= tau:
        th = np.nextafter(th, np.float32(0.0))
    while np.float32(np.float32(1.0) - th) > tau:
        th = np.nextafter(th, np.float32(2.0))
    return np.float32(th)


def _host_weights(weights, bias):
    """Fold c3 away.  Returns (wfold [2C, K2, O] f32 = [W2-W3; W4-W3],
    bias_eff [O] f32 = bias + sum_ck W3, ident_any)."""
    Wk = np.transpose(weights, (0, 1, 3, 4, 2)).reshape(O, C, K2, P)
    ident = np.all(np.abs(Wk - 1.0) <= np.float32(ATOL + RTOL), axis=-1)
    ident_any = bool(ident.any())
    Wk_eff = np.where(ident[..., None], np.float32(0.0), Wk)
    W2 = Wk_eff[:, :, :, 2].astype(np.float64)
    W3 = Wk_eff[:, :, :, 3].astype(np.float64)
    W4 = Wk_eff[:, :, :, 4].astype(np.float64)
    wfold = np.zeros((2 * C, K2, O), np.float32)
    wfold[0:C] = (W2 - W3).astype(np.float32).transpose(1, 2, 0)
    wfold[C : 2 * C] = (W4 - W3).astype(np.float32).transpose(1, 2, 0)
    bias_eff = (bias.astype(np.float64) + W3.sum(axis=(1, 2))).astype(np.float32)
    return np.ascontiguousarray(wfold), np.ascontiguousarray(bias_eff), ident_any


# ---------------------------------------------------------------- device IR


def _build_nc(theta, mode):
    import concourse.tile as tile
    from concourse import bacc, mybir

    f32 = mybir.dt.float32
    f32r = mybir.dt.float32r
    bf16 = mybir.dt.bfloat16
    Alu = mybir.AluOpType
    Act = mybir.ActivationFunctionType
    split = mode == "bf16_split"
    if mode == "float32":
        plane_dt = f32
    elif mode == "float32r":
        plane_dt = f32r
    else:
        plane_dt = bf16

    nc = bacc.Bacc("TRN2", target_bir_lowering=False, debug=False,
                   num_devices=NCORES)
    x_d = nc.dram_tensor("x", [IPC, C, H, W], f32, kind="ExternalInput").ap()
    w_d = nc.dram_tensor("wfold", [2 * C, K2, O], f32, kind="ExternalInput").ap()
    b_d = nc.dram_tensor("bias", [O, 1], f32, kind="ExternalInput").ap()
    o_d = nc.dram_tensor("out", [IPC, O, H, W], f32, kind="ExternalOutput").ap()

    with tile.TileContext(nc) as tc:
        with (
            tc.tile_pool(name="const", bufs=1) as constp,
            tc.tile_pool(name="scratch", bufs=1) as scrp,
            tc.tile_pool(name="plane", bufs=1) as planep,
            tc.tile_pool(name="ybuf", bufs=2) as ybufp,
            tc.tile_pool(name="psum", bufs=1, space="PSUM") as psump,
            tc.tile_pool(name="osb", bufs=4) as osbp,
        ):
            # ---- x loads first (phi critical path), weights after ----
            XF = scrp.tile([IPC * C, H, W], f32)      # flat x
            for i in range(IPC):
                nc.sync.dma_start(XF[i * C : (i + 1) * C], x_d[i])

            # pull the ACT table load off the critical path
            tiny = constp.tile([C, 1], f32)
            nc.vector.memset(tiny[:], 0.0)
            nc.scalar.activation(tiny[:], tiny[:], Act.Relu, bias=0.0, scale=1.0)

            # PE warmup: dummy matmuls keep HAM at K=8/8 until the real
            # stream starts (otherwise the first ~5us of matmuls run at 1.2GHz)
            zb = constp.tile([128, 512], plane_dt)
            nc.vector.memset(
                zb[:].bitcast(f32) if plane_dt == f32r else zb[:], 0.0
            )
            ps_warm = psump.tile([O, 512], f32, name="ps_warm", tag="ps0")
            for _ in range(40):
                nc.tensor.matmul(
                    ps_warm[:], zb[:, 0:128], zb[:], start=True, stop=True
                )

            # ---- weights ----
            w_sb = constp.tile([2 * C, K2, O], f32)
            nc.sync.dma_start(w_sb[:], w_d[:])
            b_sb = constp.tile([O, 1], f32)
            nc.sync.dma_start(b_sb[:], b_d[:])
            if mode == "float32":
                w_hi = w_sb
            else:
                w_hi = constp.tile([2 * C, K2, O], plane_dt)
                nc.vector.tensor_copy(w_hi[:], w_sb[:])
            if split:
                w_lo = constp.tile([2 * C, K2, O], plane_dt)
                nc.vector.tensor_tensor(w_lo[:], w_sb[:], w_hi[:], Alu.subtract)
                # lhsT1 rows: [W2h, W4h, W2h, W4h] (hi coeffs then lo coeffs)
                w_rep = constp.tile([4 * C, K2, O], plane_dt)
                nc.sync.dma_start(w_rep[0 : 2 * C], w_hi[:])
                nc.sync.dma_start(w_rep[2 * C : 4 * C], w_hi[:])
                lhs1, lhs2 = w_rep, w_lo
            else:
                lhs1, lhs2 = w_hi, None

            # ---- coefficient planes ----
            # scratch on the same partitions as each image's plane slice
            # (engine ops require equal SBUF base partitions across operands)
            RF = scrp.tile([IPC * C, H, W], f32)
            CF = scrp.tile([IPC * C, H, W], f32)

            npl = 4 if split else 2
            # plane buffers, padded layout; group order:
            #   split: [c2h, c4h, c2l, c4l]   else: [c2, c4]
            PL = [
                planep.tile([IPC * C, HP, WP], plane_dt, name=f"PL{g}")
                for g in range(npl)
            ]
            # borders: c2-like planes = 1 at v=0, everything else = 0
            # (memset rejects f32r dests; same-size bitcast to f32 is a no-op)
            for g, pl in enumerate(PL):
                bv = 1.0 if g == 0 else 0.0
                for strip in (
                    pl[:, 0, :],
                    pl[:, HP - 1, :],
                    pl[:, 1 : HP - 1, 0],
                    pl[:, 1 : HP - 1, WP - 1],
                ):
                    nc.vector.memset(
                        strip.bitcast(f32) if plane_dt == f32r else strip, bv
                    )

            def interior(pl):
                return pl[:, 1 : HP - 1, 1 : WP - 1]

            negone = constp.tile([IPC * C, 1], f32)
            nc.vector.memset(negone[:], -1.0)

            def phi_all():
                """Coefficient planes, both images at once (64 partitions)."""
                xf = XF[:]
                neg = negone[:]
                if split:
                    # bf16 rounding absorbs the isclose(v,1) mask: for
                    # v >= 1-2^-9, relu(2v-1) rounds to exactly 1.0, and the
                    # lo-plane residual lands on hi weights scaled 2^-9.
                    c2h, c4h, c2l, c4l = (interior(pl) for pl in PL)
                    nc.scalar.activation(RF[:], xf, Act.Relu, bias=neg, scale=2.0)
                    nc.vector.tensor_copy(c4h, RF[:])
                    nc.vector.tensor_tensor(c4l, RF[:], c4h, Alu.subtract)
                    nc.scalar.activation(CF[:], xf, Act.Relu, bias=1.0, scale=-2.0)
                    nc.scalar.activation(c2h, CF[:], Act.Copy)
                    nc.vector.tensor_tensor(c2l, CF[:], c2h, Alu.subtract)
                else:
                    c2, c4 = (interior(pl) for pl in PL)
                    nc.vector.tensor_scalar(
                        CF[:], xf, float(theta), None, Alu.is_ge
                    )
                    nc.scalar.activation(RF[:], xf, Act.Relu, bias=neg, scale=2.0)
                    nc.vector.tensor_tensor(RF[:], RF[:], CF[:], Alu.max)
                    nc.vector.tensor_copy(c4, RF[:])
                    nc.scalar.activation(c2, xf, Act.Relu, bias=1.0, scale=-2.0)

            phi_all()

            # ---- per-image gather + GEMM ----
            # Tap-outer loop: one LDWEIGHTS feeds 8 back-to-back matmuls
            # (same stationary operand), so drain overlaps the next fill and
            # the per-MM cost stays ~N/2.4 instead of the isolated latency.
            # All 8 L-tiles of an image accumulate in 8 PSUM banks at once.
            for i in range(IPC):
                Y = ybufp.tile([npl * C, HP, WP], plane_dt, name="Y", tag="Y")
                s = slice(i * C, (i + 1) * C)
                for g, pl in enumerate(PL):
                    nc.sync.dma_start(Y[g * C : (g + 1) * C], pl[s])

                pss = [
                    psump.tile([O, RT * W], f32, name=f"ps{t}", tag=f"ps{t}")
                    for t in range(NT)
                ]
                for ki in range(K2):
                    kh, kw = divmod(ki, KW)
                    cols = slice(kw, kw + W)
                    last = ki == K2 - 1
                    for t in range(NT):
                        rows = slice(t * RT + kh, t * RT + kh + RT)
                        nc.tensor.matmul(
                            pss[t][:], lhs1[:, ki, :], Y[:, rows, cols],
                            start=(ki == 0), stop=(last and not split),
                        )
                    if split:
                        for t in range(NT):
                            rows = slice(t * RT + kh, t * RT + kh + RT)
                            nc.tensor.matmul(
                                pss[t][:], lhs2[:, ki, :],
                                Y[0 : 2 * C, rows, cols],
                                start=False, stop=last,
                            )
                for t in range(NT):
                    osb = osbp.tile([O, RT * W], f32, name="osb")
                    if t % 2 == 0:
                        nc.scalar.activation(
                            osb[:], pss[t][:], Act.Identity, bias=b_sb[:, 0:1],
                            scale=1.0,
                        )
                    else:
                        nc.vector.tensor_scalar(
                            osb[:], pss[t][:], b_sb[:, 0:1], None, Alu.add
                        )
                    nc.sync.dma_start(
                        o_d[i, :, t * RT : (t + 1) * RT, :],
                        osb[:].rearrange("o (r w) -> o r w", r=RT),
                    )
    nc.compile()
    return nc


# ---------------------------------------------------------------- entry


def _prep(inputs):
    x = np.ascontiguousarray(np.asarray(inputs["x"], dtype=np.float32))
    weights = np.ascontiguousarray(np.asarray(inputs["weights"], dtype=np.float32))
    bias = np.ascontiguousarray(np.asarray(inputs["bias"], dtype=np.float32))
    positions = np.ascontiguousarray(
        np.asarray(inputs["positions"], dtype=np.float32)
    )
    return x, weights, bias, positions


def _fast_path_ok(x, positions):
    expect = np.linspace(-1.0, 1.0, P, dtype=np.float32)
    return (
        x.shape == (B, C, H, W)
        and positions.shape == (P,)
        and np.array_equal(positions, expect)
        and float(x.min()) >= 0.0
        and float(x.max()) <= 1.0
    )


def kernel(**inputs):
    x, weights, bias, positions = _prep(inputs)
    if not _fast_path_ok(x, positions):
        return _reference_np(x, weights, bias, positions)

    wfold, bias_eff, ident_any = _host_weights(weights, bias)
    if ident_any:
        # identity-shortcut weights present: needs the raw-v plane; use the
        # exact fallback rather than a rarely-exercised device path
        return _reference_np(x, weights, bias, positions)

    from concourse.bass_utils import run_bass_kernel_spmd

    nc = _build_nc(_compute_theta(), MODE)
    bias2d = np.ascontiguousarray(bias_eff.reshape(O, 1))
    in_maps = [
        {"x": np.ascontiguousarray(x[i * IPC : (i + 1) * IPC]),
         "wfold": wfold, "bias": bias2d}
        for i in range(NCORES)
    ]
    res = run_bass_kernel_spmd(nc, in_maps, core_ids=list(range(NCORES)))
    out = np.concatenate([res.results[i]["out"] for i in range(NCORES)], axis=0)
    return np.ascontiguousarray(out)


# ------------------------------------------------------------ dev utilities


def _run_sim(inputs):
    """CoreSim single-core run (images 0..IPC-1) for correctness debugging."""
    from concourse.bass_interp import CoreSim

    x, weights, bias, positions = _prep(inputs)
    assert _fast_path_ok(x, positions)
    wfold, bias_eff, ident_any = _host_weights(weights, bias)
    assert not ident_any
    nc = _build_nc(_compute_theta(), MODE)
    sim = CoreSim(nc)
    sim.tensor("x")[:] = x[:IPC]
    sim.tensor("wfold")[:] = wfold
    sim.tensor("bias")[:] = bias_eff.reshape(O, 1)
    sim.simulate()
    return np.array(sim.tensor("out"))


# revision 27
# speedup vs baseline: 1.0835x; 1.0391x over previous
"""Trainium2 Bass kernel for CustomPositionsPiecewiseConv2d.

Math: for knots positions=[-1,-.5,0,.5,1] and inputs x in [0,1], the active
interpolation coefficients are
    c2 = relu(1-2v),  c4 = max(relu(2v-1), T),  c3 = 1 - c2 - c4
with T = 1[v >= theta] the isclose(v,1) mask.  Since c2+c3+c4 == 1 exactly
(everywhere, including the zero-padding border), the c3 plane folds away:
    out = C2 (x) (W2-W3) + C4 (x) (W4-W3) + sum_ck W3[o,c,k] + bias
Each plane is an elementwise function of v and v is shifted/padded copies of x,
so planes are computed once per padded image and the 3x3 im2col becomes 9
shifted access-pattern reads feeding PSUM-accumulated matmuls.

Modes:
  float32       exact, 4 cyc/row on PE
  float32r      tf32-rounded operands, 1 cyc/row (err ~1.6e-4)
  float32r_split  hi/lo tf32 split of coeffs+weights; per tap one K=128 matmul
                  [c2h,c4h,c2l,c4l]x[W2h,W4h,W2h,W4h] plus one K=64 matmul
                  [c2h,c4h]x[W2l,W4l]; drops only (lo x lo) terms ~2^-24.

Sharding: data-parallel over batch, 2 images per core on 8 cores.
"""

import numpy as np

B, C, H, W = 16, 32, 64, 64
O, P, KH, KW = 128, 5, 3, 3
NCORES = 8
IPC = B // NCORES            # images per core
HP, WP = H + 2, W + 2        # padded image (pad=1)
RT = 8                       # output rows per L-tile
NT = H // RT                 # L-tiles per image
K2 = KH * KW
L = H * W
ATOL = 1e-5
RTOL = 1e-5

MODE = "bf16_split"          # float32 | float32r | bf16_split


# ---------------------------------------------------------------- host math


def _isclose_np(a, b):
    return np.abs(a - b) <= np.float32(ATOL) + np.float32(RTOL) * np.abs(b)


def _reference_np(x, weights, bias, positions):
    """Direct numpy port of the reference (fallback path)."""
    EPS = 1e-6
    Bn, Cn, Hn, Wn = x.shape
    On, _, Pn, KHn, KWn = weights.shape
    xp = np.pad(x, ((0, 0), (0, 0), (1, 1), (1, 1)))
    cols = [
        xp[:, :, i : i + Hn, j : j + Wn] for i in range(KHn) for j in range(KWn)
    ]
    pat = np.stack(cols, axis=2)
    v = pat.reshape(Bn, Cn, KHn * KWn, Hn * Wn).astype(np.float32)

    left, right = positions[:-1], positions[1:]
    denom = right - left
    denom = np.where(denom == 0, np.float32(EPS), denom)
    varc = (1.0 / denom).astype(np.float32)
    const = (-left * varc).astype(np.float32)

    m_first = _isclose_np(v, positions[0])
    m_last = _isclose_np(v, positions[-1])
    in_range = (~(m_first | m_last)) & (v >= positions[0]) & (v <= positions[-1])

    coeff = np.zeros(v.shape + (Pn,), np.float32)
    coeff[..., 0] += m_first.astype(np.float32)
    coeff[..., Pn - 1] += m_last.astype(np.float32)
    for p in range(Pn - 1):
        m = (in_range & (v >= positions[p]) & (v < positions[p + 1])).astype(
            np.float32
        )
        t = v * varc[p] + const[p]
        coeff[..., p] += m * (1.0 - t)
        coeff[..., p + 1] += m * t

    Wk = np.transpose(weights, (0, 1, 3, 4, 2)).reshape(On, Cn, KHn * KWn, Pn)
    ident = np.all(np.abs(Wk - 1.0) <= np.float32(ATOL + RTOL), axis=-1)
    Wk_eff = np.where(ident[..., None], np.float32(0.0), Wk)

    out = np.einsum("bcklp,ockp->bol", coeff, Wk_eff, optimize=True)
    out = out + np.einsum(
        "bckl,ock->bol", v, ident.astype(np.float32), optimize=True
    )
    out = out + bias[None, :, None]
    return out.reshape(Bn, On, Hn, Wn).astype(np.float32)


def _compute_theta():
    """Smallest fp32 v such that fp32(1-v) <= fp32(ATOL + RTOL*1.0), matching
    the reference's m_last = isclose(v, 1.0) for v <= 1."""
    tau = np.float32(np.float32(ATOL) + np.float32(RTOL) * np.float32(1.0))
    th = np.float32(np.float32(1.0) - tau)
    while np.float32(np.float32(1.0) - np.nextafter(th, np.float32(0.0))) <= tau:
        th = np.nextafter(th, np.float32(0.0))
    while np.float32(np.float32(1.0) - th) > tau:
        th = np.nextafter(th, np.float32(2.0))
    return np.float32(th)


def _host_weights(weights, bias):
    """Fold c3 away.  Returns (wfold [2C, K2, O] f32 = [W2-W3; W4-W3],
    bias_eff [O] f32 = bias + sum_ck W3, ident_any)."""
    Wk = np.transpose(weights, (0, 1, 3, 4, 2)).reshape(O, C, K2, P)
    ident = np.all(np.abs(Wk - 1.0) <= np.float32(ATOL + RTOL), axis=-1)
    ident_any = bool(ident.any())
    Wk_eff = np.where(ident[..., None], np.float32(0.0), Wk)
    W2 = Wk_eff[:, :, :, 2].astype(np.float64)
    W3 = Wk_eff[:, :, :, 3].astype(np.float64)
    W4 = Wk_eff[:, :, :, 4].astype(np.float64)
    wfold = np.zeros((2 * C, K2, O), np.float32)
    wfold[0:C] = (W2 - W3).astype(np.float32).transpose(1, 2, 0)
    wfold[C : 2 * C] = (W4 - W3).astype(np.float32).transpose(1, 2, 0)
    bias_eff = (bias.astype(np.float64) + W3.sum(axis=(1, 2))).astype(np.float32)
    return np.ascontiguousarray(wfold), np.ascontiguousarray(bias_eff), ident_any


# ---------------------------------------------------------------- device IR


def _build_nc(theta, mode):
    import concourse.tile as tile
    from concourse import bacc, mybir

    f32 = mybir.dt.float32
    f32r = mybir.dt.float32r
    bf16 = mybir.dt.bfloat16
    Alu = mybir.AluOpType
    Act = mybir.ActivationFunctionType
    split = mode == "bf16_split"
    if mode == "float32":
        plane_dt = f32
    elif mode == "float32r":
        plane_dt = f32r
    else:
        plane_dt = bf16

    nc = bacc.Bacc("TRN2", target_bir_lowering=False, debug=False,
                   num_devices=NCORES)
    x_d = nc.dram_tensor("x", [IPC, C, H, W], f32, kind="ExternalInput").ap()
    w_d = nc.dram_tensor("wfold", [2 * C, K2, O], f32, kind="ExternalInput").ap()
    b_d = nc.dram_tensor("bias", [O, 1], f32, kind="ExternalInput").ap()
    o_d = nc.dram_tensor("out", [IPC, O, H, W], f32, kind="ExternalOutput").ap()

    with tile.TileContext(nc) as tc:
        with (
            tc.tile_pool(name="const", bufs=1) as constp,
            tc.tile_pool(name="scratch", bufs=1) as scrp,
            tc.tile_pool(name="plane", bufs=1) as planep,
            tc.tile_pool(name="ybuf", bufs=2) as ybufp,
            tc.tile_pool(name="psum", bufs=1, space="PSUM") as psump,
            tc.tile_pool(name="osb", bufs=4) as osbp,
        ):
            # ---- x loads first (phi critical path), weights after ----
            XF = scrp.tile([IPC * C, H, W], f32)      # flat x
            for i in range(IPC):
                nc.sync.dma_start(XF[i * C : (i + 1) * C], x_d[i])

            # pull the ACT table load off the critical path
            tiny = constp.tile([C, 1], f32)
            nc.gpsimd.memset(tiny[:], 0.0)
            nc.scalar.activation(tiny[:], tiny[:], Act.Relu, bias=0.0, scale=1.0)

            # PE warmup: dummy matmuls keep HAM at K=8/8 until the real
            # stream starts (otherwise the first ~5us of matmuls run at 1.2GHz)
            zb = constp.tile([128, 512], plane_dt)
            nc.gpsimd.memset(
                zb[:].bitcast(f32) if plane_dt == f32r else zb[:], 0.0
            )
            warm_tiles = []
            for w in range(2):
                pw = psump.tile([O, 512], f32, name=f"ps_warm{w}", tag=f"ps{w}")
                warm_tiles.append(pw)
                for g in range(3):
                    for j in range(8):
                        nc.tensor.matmul(
                            pw[:], zb[:, 0:128], zb[:],
                            start=(j == 0), stop=(j == 7),
                        )

            # ---- weights ----
            w_sb = constp.tile([2 * C, K2, O], f32)
            nc.sync.dma_start(w_sb[:], w_d[:])
            b_sb = constp.tile([O, 1], f32)
            nc.sync.dma_start(b_sb[:], b_d[:])
            if mode == "float32":
                w_hi = w_sb
            else:
                w_hi = constp.tile([2 * C, K2, O], plane_dt)
                nc.vector.tensor_copy(w_hi[:], w_sb[:])
            if split:
                w_lo = constp.tile([2 * C, K2, O], plane_dt)
                nc.vector.tensor_tensor(w_lo[:], w_sb[:], w_hi[:], Alu.subtract)
                # lhsT1 rows: [W2h, W4h, W2h, W4h] (hi coeffs then lo coeffs)
                w_rep = constp.tile([4 * C, K2, O], plane_dt)
                nc.sync.dma_start(w_rep[0 : 2 * C], w_hi[:])
                nc.sync.dma_start(w_rep[2 * C : 4 * C], w_hi[:])
                lhs1, lhs2 = w_rep, w_lo
            else:
                lhs1, lhs2 = w_hi, None

            # ---- coefficient planes ----
            # scratch on the same partitions as each image's plane slice
            # (engine ops require equal SBUF base partitions across operands)
            RF = scrp.tile([IPC * C, H, W], f32)
            CF = scrp.tile([IPC * C, H, W], f32)

            npl = 4 if split else 2
            # plane buffers, padded layout; group order:
            #   split: [c2h, c4h, c2l, c4l]   else: [c2, c4]
            PL = [
                planep.tile([IPC * C, HP, WP], plane_dt, name=f"PL{g}")
                for g in range(npl)
            ]
            # borders: c2-like planes = 1 at v=0, everything else = 0
            # (memset rejects f32r dests; same-size bitcast to f32 is a no-op)
            for g, pl in enumerate(PL):
                bv = 1.0 if g == 0 else 0.0
                for strip in (
                    pl[:, 0, :],
                    pl[:, HP - 1, :],
                    pl[:, 1 : HP - 1, 0],
                    pl[:, 1 : HP - 1, WP - 1],
                ):
                    nc.gpsimd.memset(
                        strip.bitcast(f32) if plane_dt == f32r else strip, bv
                    )

            def interior(pl):
                return pl[:, 1 : HP - 1, 1 : WP - 1]

            negone = constp.tile([IPC * C, 1], f32)
            nc.gpsimd.memset(negone[:], -1.0)

            def phi_all():
                """Coefficient planes, both images at once (64 partitions)."""
                xf = XF[:]
                neg = negone[:]
                if split:
                    # bf16 rounding absorbs the isclose(v,1) mask: for
                    # v >= 1-2^-9, relu(2v-1) rounds to exactly 1.0, and the
                    # lo-plane residual lands on hi weights scaled 2^-9.
                    c2h, c4h, c2l, c4l = (interior(pl) for pl in PL)
                    nc.scalar.activation(RF[:], xf, Act.Relu, bias=neg, scale=2.0)
                    nc.vector.tensor_copy(c4h, RF[:])
                    nc.vector.tensor_tensor(c4l, RF[:], c4h, Alu.subtract)
                    nc.scalar.activation(CF[:], xf, Act.Relu, bias=1.0, scale=-2.0)
                    nc.scalar.activation(c2h, CF[:], Act.Copy)
                    nc.vector.tensor_tensor(c2l, CF[:], c2h, Alu.subtract)
                else:
                    c2, c4 = (interior(pl) for pl in PL)
                    nc.vector.tensor_scalar(
                        CF[:], xf, float(theta), None, Alu.is_ge
                    )
                    nc.scalar.activation(RF[:], xf, Act.Relu, bias=neg, scale=2.0)
                    nc.vector.tensor_tensor(RF[:], RF[:], CF[:], Alu.max)
                    nc.vector.tensor_copy(c4, RF[:])
                    nc.scalar.activation(c2, xf, Act.Relu, bias=1.0, scale=-2.0)

            phi_all()

            # ---- per-image gather + GEMM ----
            # Tap-outer loop: one LDWEIGHTS feeds 8 back-to-back matmuls
            # (same stationary operand), so drain overlaps the next fill and
            # the per-MM cost stays ~N/2.4 instead of the isolated latency.
            # All 8 L-tiles of an image accumulate in 8 PSUM banks at once.
            for i in range(IPC):
                Y = ybufp.tile([npl * C, HP, WP], plane_dt, name="Y", tag="Y")
                s = slice(i * C, (i + 1) * C)
                for g, pl in enumerate(PL):
                    nc.sync.dma_start(Y[g * C : (g + 1) * C], pl[s])

                pss = [
                    psump.tile([O, RT * W], f32, name=f"ps{t}", tag=f"ps{t}")
                    for t in range(NT)
                ]
                for ki in range(K2):
                    kh, kw = divmod(ki, KW)
                    cols = slice(kw, kw + W)
                    last = ki == K2 - 1
                    for t in range(NT):
                        rows = slice(t * RT + kh, t * RT + kh + RT)
                        nc.tensor.matmul(
                            pss[t][:], lhs1[:, ki, :], Y[:, rows, cols],
                            start=(ki == 0), stop=(last and not split),
                        )
                    if split:
                        for t in range(NT):
                            rows = slice(t * RT + kh, t * RT + kh + RT)
                            nc.tensor.matmul(
                                pss[t][:], lhs2[:, ki, :],
                                Y[0 : 2 * C, rows, cols],
                                start=False, stop=last,
                            )
                for t in range(NT):
                    osb = osbp.tile([O, RT * W], f32, name="osb")
                    if t % 2 == 0:
                        nc.scalar.activation(
                            osb[:], pss[t][:], Act.Identity, bias=b_sb[:, 0:1],
                            scale=1.0,
                        )
                    else:
                        nc.vector.tensor_scalar(
                            osb[:], pss[t][:], b_sb[:, 0:1], None, Alu.add
                        )
                    nc.sync.dma_start(
                        o_d[i, :, t * RT : (t + 1) * RT, :],
                        osb[:].rearrange("o (r w) -> o r w", r=RT),
                    )
    nc.compile()
    return nc


# ---------------------------------------------------------------- entry


def _prep(inputs):
    x = np.ascontiguousarray(np.asarray(inputs["x"], dtype=np.float32))
    weights = np.ascontiguousarray(np.asarray(inputs["weights"], dtype=np.float32))
    bias = np.ascontiguousarray(np.asarray(inputs["bias"], dtype=np.float32))
    positions = np.ascontiguousarray(
        np.asarray(inputs["positions"], dtype=np.float32)
    )
    return x, weights, bias, positions


def _fast_path_ok(x, positions):
    expect = np.linspace(-1.0, 1.0, P, dtype=np.float32)
    return (
        x.shape == (B, C, H, W)
        and positions.shape == (P,)
        and np.array_equal(positions, expect)
        and float(x.min()) >= 0.0
        and float(x.max()) <= 1.0
    )


def kernel(**inputs):
    x, weights, bias, positions = _prep(inputs)
    if not _fast_path_ok(x, positions):
        return _reference_np(x, weights, bias, positions)

    wfold, bias_eff, ident_any = _host_weights(weights, bias)
    if ident_any:
        # identity-shortcut weights present: needs the raw-v plane; use the
        # exact fallback rather than a rarely-exercised device path
        return _reference_np(x, weights, bias, positions)

    from concourse.bass_utils import run_bass_kernel_spmd

    nc = _build_nc(_compute_theta(), MODE)
    bias2d = np.ascontiguousarray(bias_eff.reshape(O, 1))
    in_maps = [
        {"x": np.ascontiguousarray(x[i * IPC : (i + 1) * IPC]),
         "wfold": wfold, "bias": bias2d}
        for i in range(NCORES)
    ]
    res = run_bass_kernel_spmd(nc, in_maps, core_ids=list(range(NCORES)))
    out = np.concatenate([res.results[i]["out"] for i in range(NCORES)], axis=0)
    return np.ascontiguousarray(out)


# ------------------------------------------------------------ dev utilities


def _run_sim(inputs):
    """CoreSim single-core run (images 0..IPC-1) for correctness debugging."""
    from concourse.bass_interp import CoreSim

    x, weights, bias, positions = _prep(inputs)
    assert _fast_path_ok(x, positions)
    wfold, bias_eff, ident_any = _host_weights(weights, bias)
    assert not ident_any
    nc = _build_nc(_compute_theta(), MODE)
    sim = CoreSim(nc)
    sim.tensor("x")[:] = x[:IPC]
    sim.tensor("wfold")[:] = wfold
    sim.tensor("bias")[:] = bias_eff.reshape(O, 1)
    sim.simulate()
    return np.array(sim.tensor("out"))
